# revision 40
# baseline (speedup 1.0000x reference)
"""Causal self-attention (B=4, T=2048, C=1024, 16 heads) on 8 TRN2 NeuronCores.

Sharding: tensor-parallel over heads. Each core owns 2 heads (128 of the
1024 q/k/v dims): wq/wk/wv are split by rows (output dim), wo by columns.
Each core computes a full [C, B*T] partial of the output projection; the
host sums the 8 partials.

On-core layout is "transposed": activations live as [feature, token] so
every matmul has tokens on the moving free dim. x and the q/k/v weights
are loaded as bf16 (host-side cast), so projections run as bf16 matmuls
(1 cycle/row); q/k are stored bf16 which also lets the diagonal-band
score matmuls shrink to their unmasked tail (bf16 has no 256-column
floor like f32r). Attention is computed as s^T = K Q^T with keys on
partitions; softmax max-subtraction is skipped (logits are O(10), exp is
safe in fp32) and the denominator comes from a ones-column appended to V
in the P^T @ V matmul. Causal masking replaces masked probabilities with
exp(-10) (the module masks logits with -10, not -inf).

Scheduling: score blocks are processed in PAIRS sharing one 2-bank PSUM
tile so each exp covers ~1024 columns (halves activation-engine
instruction count; exp'd garbage in the gaps of diagonal-band pairs is
overwritten by the affine_select fill). Heads are interleaved per query
chunk, and the output-projection matmuls of chunk c-1 are drained into
the PE stream between the score and PV matmuls of chunk c, filling the
PE stalls that the scores->exp->PV dependency chain would otherwise
leave. Output partials are staged bf16 and written with one batched DMA
per (batch, chunk); x is loaded with one DMA per [128 x 4 x 512] half.
"""

import os
import sys
from collections import deque

import numpy as np

for _p in ("/opt/trn_rl_repo",):
    if _p not in sys.path and os.path.isdir(_p):
        sys.path.insert(0, _p)

_B, _T, _C = 4, 2048, 1024
_NHEAD, _HD = 16, 64
_NC = 8
_LOC = (_NHEAD // _NC) * _HD  # feature dims per core = 128 (2 heads)
_BT = _B * _T                 # 8192 tokens
_TC = 512                     # token chunk (psum bank / moving-operand width)
_NTC = _BT // _TC             # 16 projection chunks
_KC = _C // 128               # 8 contraction chunks over the embedding
_NQC = _T // _TC              # 4 query chunks per batch
_NKB = _T // 128              # 16 key blocks per batch
_EXPM = float(np.exp(-10.0))  # exp of the mask fill value

TRACE = bool(int(os.environ.get("KERNEL_TRACE", "0")))
LAST_EXEC_NS = None
LAST_RESULTS = None

_cache = {}


def _build():
    import concourse.mybir as mybir
    import concourse.tile as tile
    from concourse import bacc

    f32 = mybir.dt.float32
    f32r = mybir.dt.float32r
    bf16 = mybir.dt.bfloat16
    AF = mybir.ActivationFunctionType

    nc = bacc.Bacc("TRN2", target_bir_lowering=False, debug=False)

    xT_d = nc.dram_tensor("xT", [_C, _BT], bf16, kind="ExternalInput").ap()
    wqT_d = nc.dram_tensor("wqT", [_C, _LOC], bf16, kind="ExternalInput").ap()
    wkT_d = nc.dram_tensor("wkT", [_C, _LOC], bf16, kind="ExternalInput").ap()
    wvT_d = nc.dram_tensor("wvT", [_C, _LOC], bf16, kind="ExternalInput").ap()
    woT_d = nc.dram_tensor("woT", [_LOC, _C], f32r, kind="ExternalInput").ap()
    idc_d = nc.dram_tensor("identc", [128, 64], f32r, kind="ExternalInput").ap()
    oneb_d = nc.dram_tensor("onesb", [128, 1], bf16, kind="ExternalInput").ap()
    oner_d = nc.dram_tensor("onesr", [1, 64], f32r, kind="ExternalInput").ap()
    outT_d = nc.dram_tensor("outT", [_C, _BT], bf16, kind="ExternalOutput").ap()

    xT_v = xT_d.rearrange("(c p) n -> p c n", p=128)    # [128, 8, 8192]
    wq_v = wqT_d.rearrange("(c p) m -> p c m", p=128)   # [128, 8, 128]
    wk_v = wkT_d.rearrange("(c p) m -> p c m", p=128)
    wv_v = wvT_d.rearrange("(c p) m -> p c m", p=128)
    wo_v = woT_d.rearrange("p (m n) -> p m n", n=128)   # [128, 8, 128]
    outT_v = outT_d.rearrange("(m p) n -> p m n", p=128)  # [128, 8, 8192]

    with tile.TileContext(nc) as tc:
        with (
            tc.tile_pool(name="consts", bufs=1) as cp,
            tc.tile_pool(name="sb", bufs=2) as sp,
            tc.tile_pool(name="ps", bufs=2, space="PSUM") as pp,
        ):
            # startup order: wq + first x chunk first so the projection
            # matmul chain starts as early as possible
            w_sb = {}
            for nm in ("q", "k", "v"):
                w_sb[nm] = cp.tile([128, _KC, 128], bf16, tag=f"w{nm}",
                                   name=f"w{nm}")
            xa0 = sp.tile([128, 4, _TC], bf16, tag="xa", bufs=2)
            xb0 = sp.tile([128, 4, _TC], bf16, tag="xb", bufs=2)

            nc.sync.dma_start(w_sb["q"][:, 0:1, :], wq_v[:, 0:1, :])
            nc.sync.dma_start(xa0[:, 0:1, :], xT_v[:, 0:1, 0:_TC])
            nc.sync.dma_start(w_sb["q"][:, 1:, :], wq_v[:, 1:, :])
            nc.sync.dma_start(xa0[:, 1:, :], xT_v[:, 1:4, 0:_TC])
            nc.sync.dma_start(xb0[:], xT_v[:, 4:8, 0:_TC])
            nc.sync.dma_start(w_sb["k"][:], wk_v[:])
            nc.sync.dma_start(w_sb["v"][:], wv_v[:])
            wo_sb = cp.tile([128, _KC, 128], f32r, tag="wo")
            nc.sync.dma_start(wo_sb[:], wo_v[:])
            ident = cp.tile([128, 64], f32r, tag="ident")
            nc.sync.dma_start(ident[:], idc_d[:])
            ones1 = cp.tile([65, 64], f32r, tag="ones1")
            nc.sync.dma_start(ones1[64:65, :], oner_d[:])
            ones_b = cp.tile([128, 1], bf16, tag="onesb")
            nc.sync.dma_start(ones_b[:], oneb_d[:])

            qT = cp.tile([128, _BT], bf16, tag="qT")
            kT = cp.tile([128, _BT], bf16, tag="kT")
            # v in [token, dim] layout per 128-token block, per head, with a
            # trailing ones column (row sums -> softmax denominator)
            vaug = cp.tile([128, _BT // 128, 2, 65], bf16, tag="vaug")
            for h in range(2):
                nc.vector.memset(vaug[:, :, h, 64:65], 1.0)

            # ---------------- q/k/v projections ----------------
            # ---------------- q/k/v projections ----------------
            # each chunk is split into 4 units (dma prefetch, q, k, v) that
            # are drained one at a time into the previous batch's attention
            # pair loop, keeping the PE fed while exp/affine/copies run
            def _proj_mm(nm, halves, out_cb, t):
                ps = pp.tile([128, _TC], f32, tag="psC", bufs=2, name="ps")
                for c in range(_KC):
                    nc.tensor.matmul(
                        ps[:],
                        w_sb[nm][:, c, :],
                        halves[c // 4][:, c % 4, :],
                        start=(c == 0),
                        stop=(c == _KC - 1),
                    )
                out_cb(ps, t)

            def _q_out(ps, t):
                nc.vector.tensor_copy(qT[:, t * _TC:(t + 1) * _TC], ps[:])

            def _k_out(ps, t):
                nc.vector.tensor_copy(kT[:, t * _TC:(t + 1) * _TC], ps[:])

            def _v_out(ps, t, st):
                vtc = sp.tile([128, _TC], f32r, tag="vtc", bufs=2)
                nc.vector.tensor_copy(vtc[:], ps[:])
                st["vtc"] = vtc

            def _v_tp(t, st):
                vtc = st["vtc"]
                for h in range(2):
                    tp = pp.tile([128, 4, 64], f32, tag="psC", bufs=2)
                    for s4 in range(4):
                        nc.tensor.transpose(
                            tp[:, s4, :].bitcast(f32r),
                            vtc[h * 64:(h + 1) * 64,
                                s4 * 128:(s4 + 1) * 128],
                            ident[h * 64:(h + 1) * 64, :],
                        )
                    nc.vector.tensor_copy(
                        vaug[:, t * 4:t * 4 + 4, h, 0:64], tp[:]
                    )

            def make_proj_units(t):
                tok = slice(t * _TC, (t + 1) * _TC)
                st = {}

                def dma_u(t=t, tok=tok, st=st):
                    if t == 0:
                        st["h"] = (xa0, xb0)
                    else:
                        xa = sp.tile([128, 4, _TC], bf16, tag="xa", bufs=2,
                                     name="xa")
                        xb = sp.tile([128, 4, _TC], bf16, tag="xb", bufs=2,
                                     name="xb")
                        nc.sync.dma_start(xa[:], xT_v[:, 0:4, tok])
                        nc.sync.dma_start(xb[:], xT_v[:, 4:8, tok])
                        st["h"] = (xa, xb)

                return dma_u, [
                    lambda t=t, st=st: _proj_mm("q", st["h"], _q_out, t),
                    lambda t=t, st=st: _proj_mm("k", st["h"], _k_out, t),
                    lambda t=t, st=st: _proj_mm(
                        "v", st["h"], lambda ps, tt: _v_out(ps, tt, st), t),
                    lambda t=t, st=st: _v_tp(t, st),
                ]

            def enqueue_chunks(ts):
                # x DMAs run two chunks ahead of their consumers (the xa/xb
                # rings hold two chunks)
                dmas, rests = [], []
                for t in ts:
                    d, r = make_proj_units(t)
                    dmas.append(d)
                    rests.append(r)
                order = [dmas[0], dmas[1]]
                for i, r in enumerate(rests):
                    order.extend(r)
                    if i + 2 < len(dmas):
                        order.append(dmas[i + 2])
                proj_q.extend(order)

            # ---------------- attention + output projection ----------------
            # pending: deferred output-projection matmuls, drained into the
            # PE stream between score and PV matmuls of later chunks
            pending = deque()
            proj_q = deque()
            credit = [0.0]
            pump_n = [0]

            def pump():
                pump_n[0] += 1
                drain_pending(2)
                credit[0] += 0.65
                while credit[0] >= 1.0 and proj_q:
                    proj_q.popleft()()
                    credit[0] -= 1.0

            def drain_pending(k, final=False):
                for _ in range(min(k, len(pending))):
                    # let the normalization that feeds a fresh pending age a
                    # couple of pumps before the PE consumes it
                    if not final and pump_n[0] - pending[0][5] < 3:
                        return
                    b_, c_, m_, ycat_, ostg_, _seq = pending.popleft()
                    cc_ = slice(c_ * _TC, (c_ + 1) * _TC)
                    ops = pp.tile([128, _TC], f32, tag="psC", bufs=2)
                    nc.tensor.matmul(
                        ops[:],
                        wo_sb[:, m_, :],
                        ycat_[:, cc_],
                        start=True, stop=True,
                    )
                    # stage to SBUF: mostly DVE, some Pool; the final batch
                    # alternates DVE/Act and splits the DMA so the drain
                    # parallelizes instead of serializing on one engine
                    if final:
                        if m_ % 2 == 0:
                            nc.scalar.copy(ostg_[:, m_, :], ops[:])
                        else:
                            nc.vector.tensor_copy(ostg_[:, m_, :], ops[:])
                        half = _KC // 2
                        if m_ == half - 1:
                            nc.sync.dma_start(
                                outT_v[:, 0:half,
                                       b_ * _T + c_ * _TC:
                                       b_ * _T + (c_ + 1) * _TC],
                                ostg_[:, 0:half, :],
                            )
                        elif m_ == _KC - 1:
                            nc.sync.dma_start(
                                outT_v[:, half:,
                                       b_ * _T + c_ * _TC:
                                       b_ * _T + (c_ + 1) * _TC],
                                ostg_[:, half:, :],
                            )
                        continue
                    nc.vector.tensor_copy(ostg_[:, m_, :], ops[:])
                    if m_ == _KC - 1:
                        nc.sync.dma_start(
                            outT_v[:, :, b_ * _T + c_ * _TC:
                                   b_ * _T + (c_ + 1) * _TC],
                            ostg_[:],
                        )

            enqueue_chunks(range(4))

            for b in range(_B):
                # finish this batch's projections (normally already drained
                # into the previous batch's attention), then queue the next
                # batch's projection units for interleaved draining
                while proj_q:
                    proj_q.popleft()()
                if b + 1 < _B:
                    enqueue_chunks(range(4 * (b + 1), 4 * (b + 1) + 4))
                ycat = sp.tile([128, _T], f32r, tag="ycat", bufs=2)
                ytmp = [sp.tile([65, _T], f32r, tag="ytmp", bufs=2,
                                name=f"ytmp{h}")
                        for h in range(2)]
                # column sums of v over each chunk's fully-masked key
                # blocks, accumulated in PSUM: suf[:, h, c] = sum over
                # kb >= 4c+4 of (v_kb^T @ 1).  Applied (scaled by
                # exp(-10)) as a bias when copying y out of PSUM.  Lives
                # briefly in a pair-pool buffer (read out immediately).
                sufh = {}

                def emit_suffix(b=b, sufh=sufh):
                    suf_ps = pp.tile([65, 2, _NQC - 1], f32, tag="pair",
                                     bufs=2)
                    for h in range(2):
                        for c in range(_NQC - 1):
                            for kb in range(4 * c + 4, _NKB):
                                nc.tensor.matmul(
                                    suf_ps[:, h, c:c + 1],
                                    vaug[:, b * 16 + kb, h, :],
                                    ones_b[:],
                                    start=(kb == 4 * c + 4),
                                    stop=(kb == _NKB - 1),
                                )
                    suf_sb = sp.tile([65, 2, _NQC - 1], f32, tag="sufsb",
                                     bufs=2)
                    nc.scalar.activation(suf_sb[:], suf_ps[:], AF.Copy,
                                         scale=_EXPM)
                    sufh["sb"] = suf_sb

                emit_suffix()

                for c in range(_NQC):
                    cc = slice(c * _TC, (c + 1) * _TC)
                    # both heads run zipped through the pair loop with
                    # separate PSUM accumulators, so every dependency
                    # (exp/affine/copy) hides behind the other head's work
                    yps = [pp.tile([65, _TC], f32, tag="yT", bufs=2,
                                   name=f"yps{h}") for h in range(2)]
                    kbs = list(range(4 * c + 4))

                    def pv_mms(pair, px, c=c, b=b):
                        for h in range(2):
                            for i, kb in enumerate(pair):
                                nc.tensor.matmul(
                                    yps[h][:],
                                    vaug[:, b * 16 + kb, h, :],
                                    px[h][:, i * _TC:(i + 1) * _TC],
                                    start=(kb == 0),
                                    stop=(kb == 4 * c + 3),
                                )

                    # PV matmuls run one pair behind the score matmuls so
                    # the scores->exp->affine->PV latency is hidden behind
                    # the next pair's scores + pumped work
                    prev_pv = None
                    for p0 in range(0, len(kbs), 2):
                        pair = kbs[p0:p0 + 2]
                        pt = [pp.tile([128, 2 * _TC], f32, tag="pair",
                                      bufs=2, name=f"pt{h}")
                              for h in range(2)]
                        px = [sp.tile([128, 2 * _TC], bf16, tag="pexp",
                                      bufs=8, name=f"px{h}")
                              for h in range(2)]
                        offs = [128 * (kb - 4 * c) if kb - 4 * c > 0 else 0
                                for kb in pair]
                        e0 = offs[0]
                        for h in range(2):
                            rows = slice(h * 64, (h + 1) * 64)
                            for i, kb in enumerate(pair):
                                off = offs[i]
                                nc.tensor.matmul(
                                    pt[h][:, i * _TC + off:(i + 1) * _TC],
                                    kT[rows,
                                       b * _T + kb * 128:
                                       b * _T + (kb + 1) * 128],
                                    qT[rows,
                                       b * _T + c * _TC + off:
                                       b * _T + (c + 1) * _TC],
                                    start=True, stop=True,
                                )
                            # exp emitted per head right behind its scores
                            # so the activation engine starts while the
                            # other head's scores still run; the gap in a
                            # j0+j1 band pair is exp'd as garbage (cheaper
                            # than a second exp) and overwritten by the
                            # affine_select fill
                            if e0 >= 256:
                                # j2+j3 band pair: the 384-column gap costs
                                # more than a second instruction
                                nc.scalar.activation(
                                    px[h][:, e0:_TC], pt[h][:, e0:_TC],
                                    AF.Exp, scale=0.125
                                )
                                o1 = _TC + offs[1]
                                nc.scalar.activation(
                                    px[h][:, o1:], pt[h][:, o1:],
                                    AF.Exp, scale=0.125
                                )
                            else:
                                nc.scalar.activation(
                                    px[h][:, e0:], pt[h][:, e0:], AF.Exp,
                                    scale=0.125
                                )
                            for i, kb in enumerate(pair):
                                j = kb - 4 * c
                                if j >= 0:
                                    # causal: keep where qi - ki - 128j
                                    # >= 0, else fill exp(-10)
                                    w = 128 * (j + 1)
                                    nc.gpsimd.affine_select(
                                        out=px[h][:, i * _TC:i * _TC + w],
                                        in_=px[h][:, i * _TC:i * _TC + w],
                                        compare_op=mybir.AluOpType.is_ge,
                                        fill=_EXPM,
                                        base=-128 * j,
                                        pattern=[[1, w]],
                                        channel_multiplier=-1,
                                    )
                        pump()
                        if prev_pv is not None:
                            pv_mms(*prev_pv)
                        prev_pv = (pair, px)
                    pump()
                    pv_mms(*prev_pv)
                    suf_sb = sufh["sb"]
                    for h in range(2):
                        if c < _NQC - 1:
                            nc.scalar.activation(
                                ytmp[h][:, cc], yps[h][:], AF.Identity,
                                bias=suf_sb[:, h, c:c + 1],
                            )
                        else:
                            nc.scalar.copy(ytmp[h][:, cc], yps[h][:])
                    pump()
                    # normalize both heads: reciprocal of the denominator
                    # row, broadcast across the 64 dims on the Pool engine
                    # (partition-stride-0 read), then multiply
                    for h in range(2):
                        rows = slice(h * 64, (h + 1) * 64)
                        zr1 = sp.tile([1, _TC], f32, tag="zr1", bufs=2)
                        nc.vector.reciprocal(zr1[:], ytmp[h][64:65, cc])
                        zb = sp.tile([64, _TC], f32, tag="zrec", bufs=2)
                        nc.gpsimd.partition_broadcast(zb[:], zr1[:])
                        nc.vector.tensor_mul(
                            ycat[rows, cc], ytmp[h][0:64, cc], zb[:]
                        )
                        pump()
                    ostg = sp.tile([128, _KC, _TC], bf16, tag="ostg",
                                   bufs=6)
                    for m in range(_KC):
                        pending.append((b, c, m, ycat, ostg, pump_n[0]))

            drain_pending(len(pending), final=True)

    nc.compile()
    return nc, outT_d.name


def _get_nc():
    if "nc" not in _cache:
        _cache["nc"] = _build()
    return _cache["nc"]


def kernel(**inputs):
    import ml_dtypes

    from concourse.bass_utils import run_bass_kernel_spmd

    bf = ml_dtypes.bfloat16
    x = np.ascontiguousarray(np.asarray(inputs["x"]), dtype=np.float32)
    wq = np.ascontiguousarray(np.asarray(inputs["wq"]), dtype=np.float32)
    wk = np.ascontiguousarray(np.asarray(inputs["wk"]), dtype=np.float32)
    wv = np.ascontiguousarray(np.asarray(inputs["wv"]), dtype=np.float32)
    wo = np.ascontiguousarray(np.asarray(inputs["wo"]), dtype=np.float32)

    xT = np.ascontiguousarray(x.reshape(_BT, _C).T.astype(bf))
    identc = np.zeros((128, 64), dtype=np.float32)
    identc[np.arange(128), np.arange(128) % 64] = 1.0
    onesb = np.ones((128, 1), dtype=bf)
    onesr = np.ones((1, 64), dtype=np.float32)

    in_maps = []
    for i in range(_NC):
        r = slice(_LOC * i, _LOC * (i + 1))
        in_maps.append({
            "xT": xT,
            "wqT": np.ascontiguousarray(wq[r].T.astype(bf)),
            "wkT": np.ascontiguousarray(wk[r].T.astype(bf)),
            "wvT": np.ascontiguousarray(wv[r].T.astype(bf)),
            "woT": np.ascontiguousarray(wo[:, r].T),
            "identc": identc,
            "onesb": onesb,
            "onesr": onesr,
        })

    nc, outname = _get_nc()
    try:
        res = run_bass_kernel_spmd(nc, in_maps, list(range(_NC)), trace=TRACE)
    except ModuleNotFoundError:
        # NTFF profiling hook unavailable in this container
        res = run_bass_kernel_spmd(nc, in_maps, list(range(_NC)), trace=False)

    global LAST_EXEC_NS, LAST_RESULTS
    LAST_EXEC_NS = res.exec_time_ns
    LAST_RESULTS = res

    acc = np.zeros((_C, _BT), dtype=np.float64)
    for i in range(_NC):
        acc += res.results[i][outname].astype(np.float64)
    return np.ascontiguousarray(acc.T).reshape(_B, _T, _C).astype(np.float32)


# revision 60
# speedup vs baseline: 1.0047x; 1.0047x over previous
"""Causal self-attention (B=4, T=2048, C=1024, 16 heads) on 8 TRN2 NeuronCores.

Sharding: tensor-parallel over heads. Each core owns 2 heads (128 of the
1024 q/k/v dims): wq/wk/wv are split by rows (output dim), wo by columns.
Each core computes a full [C, B*T] partial of the output projection; the
host sums the 8 partials (partials are written bf16, summed in f64).

On-core layout is "transposed": activations live as [feature, token] so
every matmul has tokens on the moving free dim. x and the q/k/v weights
are loaded as bf16 (host-side cast), so projections run as bf16 matmuls
(1 cycle/row); q/k are stored bf16, which lets the diagonal-band score
matmuls shrink to their unmasked tail (bf16 has no 256-column floor like
f32r). Attention is computed as s^T = K Q^T with keys on partitions;
softmax max-subtraction is skipped (logits are O(10), exp is safe in
fp32) and the denominator comes from a ones-column appended to V in the
P^T @ V matmul. Causal masking replaces masked probabilities with
exp(-10) (the module masks logits with -10, not -inf); fully-masked key
blocks are skipped and their exp(-10)*colsum(V) contribution enters as a
per-chunk bias. Normalization takes the reciprocal of the denominator
row (DVE), broadcasts it across partitions on the gpsimd engine
(partition_broadcast), and multiplies - no PE broadcast matmul.

Scheduling (tuned against the instruction-level timeline model):
- Score blocks are processed in PAIRS sharing one 2-bank PSUM tile so
  each exp covers ~1024 columns (halves activation instruction count);
  exp'd garbage in band-pair gaps is overwritten by the affine_select
  fill.  The j2+j3 band pair keeps two exps (its gap is wider than the
  instruction overhead).
- Both heads run zipped through the pair loop with separate PSUM
  accumulators; each head's exp is emitted right behind its scores so
  the activation engine starts one head while the other's scores run.
- PV matmuls trail the scores by one pair, hiding the
  scores->exp->affine->PV dependency latency.
- Projections are split into per-chunk units (dma prefetch, q, k, v,
  v-transpose) and drained one at a time into the previous batch's
  attention pair loop ("pump"), as are the deferred output-projection
  matmuls, so the PE never waits long on the exp pipeline.
- Output partials are staged bf16 and written with one batched DMA per
  (batch, chunk); x is loaded with one DMA per [128 x 4 x 512] half; the
  final batch alternates staging copies between DVE and Act and splits
  its DMAs so the tail drain parallelizes.
"""

import os
import sys
from collections import deque

import numpy as np

for _p in ("/opt/trn_rl_repo",):
    if _p not in sys.path and os.path.isdir(_p):
        sys.path.insert(0, _p)

_B, _T, _C = 4, 2048, 1024
_NHEAD, _HD = 16, 64
_NC = 8
_LOC = (_NHEAD // _NC) * _HD  # feature dims per core = 128 (2 heads)
_BT = _B * _T                 # 8192 tokens
_TC = 512                     # token chunk (psum bank / moving-operand width)
_NTC = _BT // _TC             # 16 projection chunks
_KC = _C // 128               # 8 contraction chunks over the embedding
_NQC = _T // _TC              # 4 query chunks per batch
_NKB = _T // 128              # 16 key blocks per batch
_EXPM = float(np.exp(-10.0))  # exp of the mask fill value

TRACE = bool(int(os.environ.get("KERNEL_TRACE", "0")))
LAST_EXEC_NS = None
LAST_RESULTS = None

_cache = {}


def _build():
    import concourse.mybir as mybir
    import concourse.tile as tile
    from concourse import bacc

    f32 = mybir.dt.float32
    f32r = mybir.dt.float32r
    bf16 = mybir.dt.bfloat16
    AF = mybir.ActivationFunctionType

    nc = bacc.Bacc("TRN2", target_bir_lowering=False, debug=False)

    xT_d = nc.dram_tensor("xT", [_C, _BT], bf16, kind="ExternalInput").ap()
    wqT_d = nc.dram_tensor("wqT", [_C, _LOC], bf16, kind="ExternalInput").ap()
    wkT_d = nc.dram_tensor("wkT", [_C, _LOC], bf16, kind="ExternalInput").ap()
    wvT_d = nc.dram_tensor("wvT", [_C, _LOC], bf16, kind="ExternalInput").ap()
    woT_d = nc.dram_tensor("woT", [_LOC, _C], f32r, kind="ExternalInput").ap()
    idc_d = nc.dram_tensor("identc", [128, 64], f32r, kind="ExternalInput").ap()
    oneb_d = nc.dram_tensor("onesb", [128, 1], bf16, kind="ExternalInput").ap()
    outT_d = nc.dram_tensor("outT", [_C, _BT], bf16, kind="ExternalOutput").ap()

    xT_v = xT_d.rearrange("(c p) n -> p c n", p=128)    # [128, 8, 8192]
    wq_v = wqT_d.rearrange("(c p) m -> p c m", p=128)   # [128, 8, 128]
    wk_v = wkT_d.rearrange("(c p) m -> p c m", p=128)
    wv_v = wvT_d.rearrange("(c p) m -> p c m", p=128)
    wo_v = woT_d.rearrange("p (m n) -> p m n", n=128)   # [128, 8, 128]
    outT_v = outT_d.rearrange("(m p) n -> p m n", p=128)  # [128, 8, 8192]

    with tile.TileContext(nc) as tc:
        with (
            tc.tile_pool(name="consts", bufs=1) as cp,
            tc.tile_pool(name="sb", bufs=2) as sp,
            tc.tile_pool(name="ps", bufs=2, space="PSUM") as pp,
        ):
            # startup order: wq + first x chunk first so the projection
            # matmul chain starts as early as possible
            w_sb = {}
            for nm in ("q", "k", "v"):
                w_sb[nm] = cp.tile([128, _KC, 128], bf16, tag=f"w{nm}",
                                   name=f"w{nm}")
            xa0 = sp.tile([128, 4, _TC], bf16, tag="xa", bufs=3)
            xb0 = sp.tile([128, 4, _TC], bf16, tag="xb", bufs=3)

            nc.sync.dma_start(w_sb["q"][:, 0:1, :], wq_v[:, 0:1, :])
            nc.sync.dma_start(xa0[:, 0:1, :], xT_v[:, 0:1, 0:_TC])
            nc.sync.dma_start(w_sb["q"][:, 1:, :], wq_v[:, 1:, :])
            nc.sync.dma_start(xa0[:, 1:, :], xT_v[:, 1:4, 0:_TC])
            nc.sync.dma_start(xb0[:], xT_v[:, 4:8, 0:_TC])
            nc.sync.dma_start(w_sb["k"][:], wk_v[:])
            nc.sync.dma_start(w_sb["v"][:], wv_v[:])
            wo_sb = cp.tile([128, _KC, 128], f32r, tag="wo")
            nc.sync.dma_start(wo_sb[:], wo_v[:])
            ident = cp.tile([128, 64], f32r, tag="ident")
            nc.sync.dma_start(ident[:], idc_d[:])
            ones_b = cp.tile([128, 1], bf16, tag="onesb")
            nc.sync.dma_start(ones_b[:], oneb_d[:])

            qT = cp.tile([128, _BT], bf16, tag="qT")
            kT = cp.tile([128, _BT], bf16, tag="kT")
            # v in [token, dim] layout per 128-token block, per head, with a
            # trailing ones column (row sums -> softmax denominator)
            vaug = cp.tile([128, _BT // 128, 2, 65], bf16, tag="vaug")
            for h in range(2):
                nc.vector.memset(vaug[:, :, h, 64:65], 1.0)

            # ---------------- q/k/v projections ----------------
            # each chunk is split into 4 units (dma prefetch, q, k, v) that
            # are drained one at a time into the previous batch's attention
            # pair loop, keeping the PE fed while exp/affine/copies run
            def _proj_mm(nm, halves, out_cb, t):
                ps = pp.tile([128, _TC], f32, tag="psC", bufs=2, name="ps")
                for c in range(_KC):
                    nc.tensor.matmul(
                        ps[:],
                        w_sb[nm][:, c, :],
                        halves[c // 4][:, c % 4, :],
                        start=(c == 0),
                        stop=(c == _KC - 1),
                    )
                out_cb(ps, t)

            def _q_out(ps, t):
                nc.vector.tensor_copy(qT[:, t * _TC:(t + 1) * _TC], ps[:])

            def _k_out(ps, t):
                nc.vector.tensor_copy(kT[:, t * _TC:(t + 1) * _TC], ps[:])

            def _v_out(ps, t, st):
                vtc = sp.tile([128, _TC], f32r, tag="vtc", bufs=2)
                nc.vector.tensor_copy(vtc[:], ps[:])
                st["vtc"] = vtc

            def _v_tp(t, st):
                vtc = st["vtc"]
                for h in range(2):
                    tp = pp.tile([128, 4, 64], f32, tag="psC", bufs=2)
                    for s4 in range(4):
                        nc.tensor.transpose(
                            tp[:, s4, :].bitcast(f32r),
                            vtc[h * 64:(h + 1) * 64,
                                s4 * 128:(s4 + 1) * 128],
                            ident[h * 64:(h + 1) * 64, :],
                        )
                    nc.vector.tensor_copy(
                        vaug[:, t * 4:t * 4 + 4, h, 0:64], tp[:]
                    )

            def make_proj_units(t):
                tok = slice(t * _TC, (t + 1) * _TC)
                st = {}

                def dma_u(t=t, tok=tok, st=st):
                    if t == 0:
                        st["h"] = (xa0, xb0)
                    else:
                        xa = sp.tile([128, 4, _TC], bf16, tag="xa", bufs=3,
                                     name="xa")
                        xb = sp.tile([128, 4, _TC], bf16, tag="xb", bufs=3,
                                     name="xb")
                        nc.sync.dma_start(xa[:], xT_v[:, 0:4, tok])
                        nc.sync.dma_start(xb[:], xT_v[:, 4:8, tok])
                        st["h"] = (xa, xb)

                return dma_u, [
                    lambda t=t, st=st: _proj_mm("q", st["h"], _q_out, t),
                    lambda t=t, st=st: _proj_mm("k", st["h"], _k_out, t),
                    lambda t=t, st=st: _proj_mm(
                        "v", st["h"], lambda ps, tt: _v_out(ps, tt, st), t),
                    lambda t=t, st=st: _v_tp(t, st),
                ]

            def enqueue_chunks(ts):
                # x DMAs run two chunks ahead of their consumers (the xa/xb
                # rings hold two chunks)
                dmas, rests = [], []
                for t in ts:
                    d, r = make_proj_units(t)
                    dmas.append(d)
                    rests.append(r)
                order = [dmas[0], dmas[1]]
                for i, r in enumerate(rests):
                    order.extend(r)
                    if i + 2 < len(dmas):
                        order.append(dmas[i + 2])
                proj_q.extend(order)

            # ---------------- attention + output projection ----------------
            # pending: deferred output-projection matmuls, drained into the
            # PE stream between score and PV matmuls of later chunks
            pending = deque()
            proj_q = deque()
            credit = [0.0]
            pump_n = [0]

            def pump():
                pump_n[0] += 1
                drain_pending(2)
                credit[0] += 0.65
                while credit[0] >= 1.0 and proj_q:
                    proj_q.popleft()()
                    credit[0] -= 1.0

            def drain_pending(k, final=False):
                for _ in range(min(k, len(pending))):
                    # let the normalization that feeds a fresh pending age a
                    # couple of pumps before the PE consumes it
                    if not final and pump_n[0] - pending[0][5] < 2:
                        return
                    b_, c_, m_, ycat_, ostg_, _seq = pending.popleft()
                    cc_ = slice(c_ * _TC, (c_ + 1) * _TC)
                    ops = pp.tile([128, _TC], f32, tag="psC", bufs=2)
                    nc.tensor.matmul(
                        ops[:],
                        wo_sb[:, m_, :],
                        ycat_[:, cc_],
                        start=True, stop=True,
                    )
                    # the final batch alternates DVE/Act staging copies and
                    # splits the DMA so the tail drain parallelizes
                    if final:
                        if m_ % 2 == 0:
                            nc.scalar.copy(ostg_[:, m_, :], ops[:])
                        else:
                            nc.vector.tensor_copy(ostg_[:, m_, :], ops[:])
                        half = _KC // 2
                        if m_ == half - 1:
                            nc.sync.dma_start(
                                outT_v[:, 0:half,
                                       b_ * _T + c_ * _TC:
                                       b_ * _T + (c_ + 1) * _TC],
                                ostg_[:, 0:half, :],
                            )
                        elif m_ == _KC - 1:
                            nc.sync.dma_start(
                                outT_v[:, half:,
                                       b_ * _T + c_ * _TC:
                                       b_ * _T + (c_ + 1) * _TC],
                                ostg_[:, half:, :],
                            )
                        continue
                    nc.vector.tensor_copy(ostg_[:, m_, :], ops[:])
                    if m_ == _KC - 1:
                        nc.sync.dma_start(
                            outT_v[:, :, b_ * _T + c_ * _TC:
                                   b_ * _T + (c_ + 1) * _TC],
                            ostg_[:],
                        )

            enqueue_chunks(range(4))

            for b in range(_B):
                # finish this batch's projections (normally already drained
                # into the previous batch's attention), then queue the next
                # batch's projection units for interleaved draining
                while proj_q:
                    proj_q.popleft()()
                if b + 1 < _B:
                    enqueue_chunks(range(4 * (b + 1), 4 * (b + 1) + 4))
                ycat = sp.tile([128, _T], f32r, tag="ycat", bufs=2)
                ytmp = [sp.tile([65, _T], f32r, tag="ytmp", bufs=2,
                                name=f"ytmp{h}")
                        for h in range(2)]
                # column sums of v over each chunk's fully-masked key
                # blocks, accumulated in PSUM: suf[:, h, c] = sum over
                # kb >= 4c+4 of (v_kb^T @ 1).  Applied (scaled by
                # exp(-10)) as a bias when copying y out of PSUM.  Lives
                # briefly in a pair-pool buffer (read out immediately).
                sufh = {}

                def emit_suffix(b=b, sufh=sufh):
                    suf_ps = pp.tile([65, 2, _NQC - 1], f32, tag="pair",
                                     bufs=2)
                    for h in range(2):
                        for c in range(_NQC - 1):
                            for kb in range(4 * c + 4, _NKB):
                                nc.tensor.matmul(
                                    suf_ps[:, h, c:c + 1],
                                    vaug[:, b * 16 + kb, h, :],
                                    ones_b[:],
                                    start=(kb == 4 * c + 4),
                                    stop=(kb == _NKB - 1),
                                )
                    suf_sb = sp.tile([65, 2, _NQC - 1], f32, tag="sufsb",
                                     bufs=2)
                    nc.scalar.activation(suf_sb[:], suf_ps[:], AF.Copy,
                                         scale=_EXPM)
                    sufh["sb"] = suf_sb

                emit_suffix()

                for c in range(_NQC):
                    cc = slice(c * _TC, (c + 1) * _TC)
                    # both heads run zipped through the pair loop with
                    # separate PSUM accumulators, so every dependency
                    # (exp/affine/copy) hides behind the other head's work
                    yps = [pp.tile([65, _TC], f32, tag="yT", bufs=2,
                                   name=f"yps{h}") for h in range(2)]
                    kbs = list(range(4 * c + 4))

                    def pv_mms(pair, px, c=c, b=b):
                        for h in range(2):
                            for i, kb in enumerate(pair):
                                nc.tensor.matmul(
                                    yps[h][:],
                                    vaug[:, b * 16 + kb, h, :],
                                    px[h][:, i * _TC:(i + 1) * _TC],
                                    start=(kb == 0),
                                    stop=(kb == 4 * c + 3),
                                )

                    # PV matmuls run one pair behind the score matmuls so
                    # the scores->exp->affine->PV latency is hidden behind
                    # the next pair's scores + pumped work
                    prev_pv = None
                    for p0 in range(0, len(kbs), 2):
                        pair = kbs[p0:p0 + 2]
                        pt = [pp.tile([128, 2 * _TC], f32, tag="pair",
                                      bufs=2, name=f"pt{h}")
                              for h in range(2)]
                        px = [sp.tile([128, 2 * _TC], bf16, tag="pexp",
                                      bufs=12, name=f"px{h}")
                              for h in range(2)]
                        offs = [128 * (kb - 4 * c) if kb - 4 * c > 0 else 0
                                for kb in pair]
                        e0 = offs[0]
                        for h in range(2):
                            rows = slice(h * 64, (h + 1) * 64)
                            for i, kb in enumerate(pair):
                                off = offs[i]
                                nc.tensor.matmul(
                                    pt[h][:, i * _TC + off:(i + 1) * _TC],
                                    kT[rows,
                                       b * _T + kb * 128:
                                       b * _T + (kb + 1) * 128],
                                    qT[rows,
                                       b * _T + c * _TC + off:
                                       b * _T + (c + 1) * _TC],
                                    start=True, stop=True,
                                )
                            # exp emitted per head right behind its scores
                            # so the activation engine starts while the
                            # other head's scores still run; the gap in a
                            # j0+j1 band pair is exp'd as garbage (cheaper
                            # than a second exp) and overwritten by the
                            # affine_select fill
                            if e0 >= 256:
                                # j2+j3 band pair: the 384-column gap costs
                                # more than a second instruction
                                nc.scalar.activation(
                                    px[h][:, e0:_TC], pt[h][:, e0:_TC],
                                    AF.Exp, scale=0.125
                                )
                                o1 = _TC + offs[1]
                                nc.scalar.activation(
                                    px[h][:, o1:], pt[h][:, o1:],
                                    AF.Exp, scale=0.125
                                )
                            else:
                                nc.scalar.activation(
                                    px[h][:, e0:], pt[h][:, e0:], AF.Exp,
                                    scale=0.125
                                )
                            for i, kb in enumerate(pair):
                                j = kb - 4 * c
                                if j >= 0:
                                    # causal: keep where qi - ki - 128j
                                    # >= 0, else fill exp(-10)
                                    w = 128 * (j + 1)
                                    nc.gpsimd.affine_select(
                                        out=px[h][:, i * _TC:i * _TC + w],
                                        in_=px[h][:, i * _TC:i * _TC + w],
                                        compare_op=mybir.AluOpType.is_ge,
                                        fill=_EXPM,
                                        base=-128 * j,
                                        pattern=[[1, w]],
                                        channel_multiplier=-1,
                                    )
                        pump()
                        if prev_pv is not None:
                            pv_mms(*prev_pv)
                        prev_pv = (pair, px)
                    pump()
                    pv_mms(*prev_pv)
                    suf_sb = sufh["sb"]
                    for h in range(2):
                        if c < _NQC - 1:
                            nc.scalar.activation(
                                ytmp[h][:, cc], yps[h][:], AF.Identity,
                                bias=suf_sb[:, h, c:c + 1],
                            )
                        else:
                            nc.scalar.copy(ytmp[h][:, cc], yps[h][:])
                    pump()
                    # normalize both heads: reciprocal of the denominator
                    # row, broadcast across the 64 dims on the Pool engine
                    # (partition-stride-0 read), then multiply
                    for h in range(2):
                        rows = slice(h * 64, (h + 1) * 64)
                        zr1 = sp.tile([1, _TC], f32, tag="zr1", bufs=2)
                        nc.vector.reciprocal(zr1[:], ytmp[h][64:65, cc])
                        zb = sp.tile([64, _TC], f32, tag="zrec", bufs=2)
                        nc.gpsimd.partition_broadcast(zb[:], zr1[:])
                        nc.vector.tensor_mul(
                            ycat[rows, cc], ytmp[h][0:64, cc], zb[:]
                        )
                        pump()
                    ostg = sp.tile([128, _KC, _TC], bf16, tag="ostg",
                                   bufs=6)
                    for m in range(_KC):
                        pending.append((b, c, m, ycat, ostg, pump_n[0]))

            drain_pending(len(pending), final=True)

    nc.compile()
    return nc, outT_d.name


def _get_nc():
    if "nc" not in _cache:
        _cache["nc"] = _build()
    return _cache["nc"]


def kernel(**inputs):
    import ml_dtypes

    from concourse.bass_utils import run_bass_kernel_spmd

    bf = ml_dtypes.bfloat16
    x = np.ascontiguousarray(np.asarray(inputs["x"]), dtype=np.float32)
    wq = np.ascontiguousarray(np.asarray(inputs["wq"]), dtype=np.float32)
    wk = np.ascontiguousarray(np.asarray(inputs["wk"]), dtype=np.float32)
    wv = np.ascontiguousarray(np.asarray(inputs["wv"]), dtype=np.float32)
    wo = np.ascontiguousarray(np.asarray(inputs["wo"]), dtype=np.float32)

    xT = np.ascontiguousarray(x.reshape(_BT, _C).T.astype(bf))
    identc = np.zeros((128, 64), dtype=np.float32)
    identc[np.arange(128), np.arange(128) % 64] = 1.0
    onesb = np.ones((128, 1), dtype=bf)

    in_maps = []
    for i in range(_NC):
        r = slice(_LOC * i, _LOC * (i + 1))
        in_maps.append({
            "xT": xT,
            "wqT": np.ascontiguousarray(wq[r].T.astype(bf)),
            "wkT": np.ascontiguousarray(wk[r].T.astype(bf)),
            "wvT": np.ascontiguousarray(wv[r].T.astype(bf)),
            "woT": np.ascontiguousarray(wo[:, r].T),
            "identc": identc,
            "onesb": onesb,
        })

    nc, outname = _get_nc()
    try:
        res = run_bass_kernel_spmd(nc, in_maps, list(range(_NC)), trace=TRACE)
    except ModuleNotFoundError:
        # NTFF profiling hook unavailable in this container
        res = run_bass_kernel_spmd(nc, in_maps, list(range(_NC)), trace=False)

    global LAST_EXEC_NS, LAST_RESULTS
    LAST_EXEC_NS = res.exec_time_ns
    LAST_RESULTS = res

    acc = np.zeros((_C, _BT), dtype=np.float64)
    for i in range(_NC):
        acc += res.results[i][outname].astype(np.float64)
    return np.ascontiguousarray(acc.T).reshape(_B, _T, _C).astype(np.float32)


# revision 74
# speedup vs baseline: 1.0160x; 1.0112x over previous
"""Causal self-attention (B=4, T=2048, C=1024, 16 heads) on 8 TRN2 NeuronCores.

Sharding: tensor-parallel over heads. Each core owns 2 heads (128 of the
1024 q/k/v dims): wq/wk/wv are split by rows (output dim), wo by columns.
Each core computes a full [C, B*T] partial of the output projection; the
host sums the 8 partials (partials are written bf16, summed in f64).

On-core layout is "transposed": activations live as [feature, token] so
every matmul has tokens on the moving free dim. x and the q/k/v weights
are loaded as bf16 (host-side cast), so projections run as bf16 matmuls
(1 cycle/row); q/k are stored bf16, which lets the diagonal-band score
matmuls shrink to their unmasked tail (bf16 has no 256-column floor like
f32r). Attention is computed as s^T = K Q^T with keys on partitions;
softmax max-subtraction is skipped (logits are O(10), exp is safe in
fp32) and the denominator comes from a ones-column appended to V in the
P^T @ V matmul. Causal masking replaces masked probabilities with
exp(-10) (the module masks logits with -10, not -inf); fully-masked key
blocks are skipped and their exp(-10)*colsum(V) contribution enters as a
per-chunk bias. Normalization takes the reciprocal of the denominator
row (DVE), broadcasts it across partitions on the gpsimd engine
(partition_broadcast), and multiplies - no PE broadcast matmul.

Scheduling (tuned against the instruction-level timeline model):
- Score blocks are processed in PAIRS sharing one 2-bank PSUM tile so
  each exp covers ~1024 columns (halves activation instruction count);
  exp'd garbage in band-pair gaps is overwritten by the affine_select
  fill.  The j2+j3 band pair keeps two exps (its gap is wider than the
  instruction overhead).
- Both heads run zipped through the pair loop with separate PSUM
  accumulators; each head's exp is emitted right behind its scores so
  the activation engine starts one head while the other's scores run.
- PV matmuls trail the scores by one pair, hiding the
  scores->exp->affine->PV dependency latency.
- Projections are split into per-chunk units (dma prefetch, q, k, v,
  v-transpose) and drained one at a time into the previous batch's
  attention pair loop ("pump"), as are the deferred output-projection
  matmuls, so the PE never waits long on the exp pipeline.
- Output partials are staged bf16 and written with one batched DMA per
  (batch, chunk); x is loaded with one DMA per [128 x 4 x 512] half; the
  final batch alternates staging copies between DVE and Act and splits
  its DMAs so the tail drain parallelizes.
"""

import os
import sys
from collections import deque

import numpy as np

for _p in ("/opt/trn_rl_repo",):
    if _p not in sys.path and os.path.isdir(_p):
        sys.path.insert(0, _p)

_B, _T, _C = 4, 2048, 1024
_NHEAD, _HD = 16, 64
_NC = 8
_LOC = (_NHEAD // _NC) * _HD  # feature dims per core = 128 (2 heads)
_BT = _B * _T                 # 8192 tokens
_TC = 512                     # token chunk (psum bank / moving-operand width)
_NTC = _BT // _TC             # 16 projection chunks
_KC = _C // 128               # 8 contraction chunks over the embedding
_NQC = _T // _TC              # 4 query chunks per batch
_NKB = _T // 128              # 16 key blocks per batch
_EXPM = float(np.exp(-10.0))  # exp of the mask fill value

TRACE = bool(int(os.environ.get("KERNEL_TRACE", "0")))
LAST_EXEC_NS = None
LAST_RESULTS = None

_cache = {}


def _build():
    import concourse.mybir as mybir
    import concourse.tile as tile
    from concourse import bacc

    f32 = mybir.dt.float32
    f32r = mybir.dt.float32r
    bf16 = mybir.dt.bfloat16
    AF = mybir.ActivationFunctionType

    nc = bacc.Bacc("TRN2", target_bir_lowering=False, debug=False)

    xT_d = nc.dram_tensor("xT", [_C, _BT], bf16, kind="ExternalInput").ap()
    wqT_d = nc.dram_tensor("wqT", [_C, _LOC], bf16, kind="ExternalInput").ap()
    wkT_d = nc.dram_tensor("wkT", [_C, _LOC], bf16, kind="ExternalInput").ap()
    wvT_d = nc.dram_tensor("wvT", [_C, _LOC], bf16, kind="ExternalInput").ap()
    woT_d = nc.dram_tensor("woT", [_LOC, _C], f32r, kind="ExternalInput").ap()
    idc_d = nc.dram_tensor("identc", [128, 64], f32r, kind="ExternalInput").ap()
    oneb_d = nc.dram_tensor("onesb", [128, 1], bf16, kind="ExternalInput").ap()
    outT_d = nc.dram_tensor("outT", [_C, _BT], bf16, kind="ExternalOutput").ap()

    xT_v = xT_d.rearrange("(c p) n -> p c n", p=128)    # [128, 8, 8192]
    wq_v = wqT_d.rearrange("(c p) m -> p c m", p=128)   # [128, 8, 128]
    wk_v = wkT_d.rearrange("(c p) m -> p c m", p=128)
    wv_v = wvT_d.rearrange("(c p) m -> p c m", p=128)
    wo_v = woT_d.rearrange("p (m n) -> p m n", n=128)   # [128, 8, 128]
    outT_v = outT_d.rearrange("(m p) n -> p m n", p=128)  # [128, 8, 8192]

    with tile.TileContext(nc) as tc:
        with (
            tc.tile_pool(name="consts", bufs=1) as cp,
            tc.tile_pool(name="sb", bufs=2) as sp,
            tc.tile_pool(name="ps", bufs=2, space="PSUM") as pp,
        ):
            # startup order: wq + first x chunk first so the projection
            # matmul chain starts as early as possible
            w_sb = {}
            for nm in ("q", "k", "v"):
                w_sb[nm] = cp.tile([128, _KC, 128], bf16, tag=f"w{nm}",
                                   name=f"w{nm}")
            xa0 = sp.tile([128, 4, _TC], bf16, tag="xa", bufs=3)
            xb0 = sp.tile([128, 4, _TC], bf16, tag="xb", bufs=3)

            nc.sync.dma_start(w_sb["q"][:, 0:1, :], wq_v[:, 0:1, :])
            nc.sync.dma_start(xa0[:, 0:1, :], xT_v[:, 0:1, 0:_TC])
            nc.sync.dma_start(w_sb["q"][:, 1:, :], wq_v[:, 1:, :])
            nc.sync.dma_start(xa0[:, 1:, :], xT_v[:, 1:4, 0:_TC])
            nc.sync.dma_start(xb0[:], xT_v[:, 4:8, 0:_TC])
            nc.sync.dma_start(w_sb["k"][:], wk_v[:])
            nc.sync.dma_start(w_sb["v"][:], wv_v[:])
            wo_sb = cp.tile([128, _KC, 128], f32r, tag="wo")
            nc.sync.dma_start(wo_sb[:], wo_v[:])
            ident = cp.tile([128, 64], f32r, tag="ident")
            nc.sync.dma_start(ident[:], idc_d[:])
            ones_b = cp.tile([128, 1], bf16, tag="onesb")
            nc.sync.dma_start(ones_b[:], oneb_d[:])

            qT = cp.tile([128, _BT], bf16, tag="qT")
            kT = cp.tile([128, _BT], bf16, tag="kT")
            # v in [token, dim] layout per 128-token block, per head, with a
            # trailing ones column (row sums -> softmax denominator)
            vaug = cp.tile([128, _BT // 128, 2, 65], bf16, tag="vaug")
            for h in range(2):
                nc.vector.memset(vaug[:, :, h, 64:65], 1.0)

            # ---------------- q/k/v projections ----------------
            # each chunk is split into 4 units (dma prefetch, q, k, v) that
            # are drained one at a time into the previous batch's attention
            # pair loop, keeping the PE fed while exp/affine/copies run
            def _proj_mm(nm, halves, out_cb, t):
                ps = pp.tile([128, _TC], f32, tag="psC", bufs=2, name="ps")
                for c in range(_KC):
                    nc.tensor.matmul(
                        ps[:],
                        w_sb[nm][:, c, :],
                        halves[c // 4][:, c % 4, :],
                        start=(c == 0),
                        stop=(c == _KC - 1),
                    )
                out_cb(ps, t)

            def _q_out(ps, t):
                nc.vector.tensor_copy(qT[:, t * _TC:(t + 1) * _TC], ps[:])

            def _k_out(ps, t):
                nc.vector.tensor_copy(kT[:, t * _TC:(t + 1) * _TC], ps[:])

            def _v_out(ps, t, st):
                vtc = sp.tile([128, _TC], f32r, tag="vtc", bufs=2)
                nc.vector.tensor_copy(vtc[:], ps[:])
                st["vtc"] = vtc

            def _v_tp(t, st):
                vtc = st["vtc"]
                for h in range(2):
                    tp = pp.tile([128, 4, 64], f32, tag="psC", bufs=2)
                    for s4 in range(4):
                        nc.tensor.transpose(
                            tp[:, s4, :].bitcast(f32r),
                            vtc[h * 64:(h + 1) * 64,
                                s4 * 128:(s4 + 1) * 128],
                            ident[h * 64:(h + 1) * 64, :],
                        )
                    nc.vector.tensor_copy(
                        vaug[:, t * 4:t * 4 + 4, h, 0:64], tp[:]
                    )

            def make_proj_units(t):
                tok = slice(t * _TC, (t + 1) * _TC)
                st = {}

                def dma_u(t=t, tok=tok, st=st):
                    if t == 0:
                        st["h"] = (xa0, xb0)
                    else:
                        xa = sp.tile([128, 4, _TC], bf16, tag="xa", bufs=3,
                                     name="xa")
                        xb = sp.tile([128, 4, _TC], bf16, tag="xb", bufs=3,
                                     name="xb")
                        nc.sync.dma_start(xa[:], xT_v[:, 0:4, tok])
                        nc.sync.dma_start(xb[:], xT_v[:, 4:8, tok])
                        st["h"] = (xa, xb)

                return dma_u, [
                    lambda t=t, st=st: _proj_mm("q", st["h"], _q_out, t),
                    lambda t=t, st=st: _proj_mm("k", st["h"], _k_out, t),
                    lambda t=t, st=st: _proj_mm(
                        "v", st["h"], lambda ps, tt: _v_out(ps, tt, st), t),
                    lambda t=t, st=st: _v_tp(t, st),
                ]

            def enqueue_chunks(ts):
                # x DMAs run two chunks ahead of their consumers (the xa/xb
                # rings hold two chunks)
                dmas, rests = [], []
                for t in ts:
                    d, r = make_proj_units(t)
                    dmas.append(d)
                    rests.append(r)
                # the last chunk's v units drain before its q/k: the next
                # batch's suffix matmuls need the whole vaug, while the q/k
                # consumers (its chunk-3 attention) are far away
                order = [dmas[0], dmas[1]]
                for i, r in enumerate(rests):
                    if i >= len(rests) - 2:
                        order.extend(r[2:4] + r[0:2])
                    else:
                        order.extend(r)
                    if i + 2 < len(dmas):
                        order.append(dmas[i + 2])
                proj_q.extend(order)

            # ---------------- attention + output projection ----------------
            # pending: deferred output-projection matmuls, drained into the
            # PE stream between score and PV matmuls of later chunks
            pending = deque()
            proj_q = deque()
            credit = [0.0]
            pump_n = [0]

            def pump():
                pump_n[0] += 1
                drain_pending(2)
                credit[0] += 0.65
                while credit[0] >= 1.0 and proj_q:
                    proj_q.popleft()()
                    credit[0] -= 1.0

            def drain_pending(k, final=False):
                for _ in range(min(k, len(pending))):
                    # let the normalization that feeds a fresh pending age a
                    # couple of pumps before the PE consumes it
                    if not final and pump_n[0] - pending[0][5] < 2:
                        return
                    b_, c_, m_, ycat_, ostg_, _seq = pending.popleft()
                    cc_ = slice(c_ * _TC, (c_ + 1) * _TC)
                    ops = pp.tile([128, _TC], f32, tag="psC", bufs=2)
                    nc.tensor.matmul(
                        ops[:],
                        wo_sb[:, m_, :],
                        ycat_[:, cc_],
                        start=True, stop=True,
                    )
                    # the final batch alternates DVE/Act staging copies and
                    # splits the DMA so the tail drain parallelizes
                    if final:
                        if m_ % 2 == 0:
                            nc.scalar.copy(ostg_[:, m_, :], ops[:])
                        else:
                            nc.vector.tensor_copy(ostg_[:, m_, :], ops[:])
                        half = _KC // 2
                        if m_ == half - 1:
                            nc.sync.dma_start(
                                outT_v[:, 0:half,
                                       b_ * _T + c_ * _TC:
                                       b_ * _T + (c_ + 1) * _TC],
                                ostg_[:, 0:half, :],
                            )
                        elif m_ == _KC - 1:
                            nc.sync.dma_start(
                                outT_v[:, half:,
                                       b_ * _T + c_ * _TC:
                                       b_ * _T + (c_ + 1) * _TC],
                                ostg_[:, half:, :],
                            )
                        continue
                    nc.vector.tensor_copy(ostg_[:, m_, :], ops[:])
                    if m_ == _KC - 1:
                        nc.sync.dma_start(
                            outT_v[:, :, b_ * _T + c_ * _TC:
                                   b_ * _T + (c_ + 1) * _TC],
                            ostg_[:],
                        )

            enqueue_chunks(range(4))

            for b in range(_B):
                # finish this batch's projections (normally already drained
                # into the previous batch's attention), then queue the next
                # batch's projection units for interleaved draining
                while proj_q:
                    proj_q.popleft()()
                if b + 1 < _B:
                    enqueue_chunks(range(4 * (b + 1), 4 * (b + 1) + 4))
                ycat = sp.tile([128, _T], f32r, tag="ycat", bufs=2)
                ytmp = [sp.tile([65, _T], f32r, tag="ytmp", bufs=2,
                                name=f"ytmp{h}")
                        for h in range(2)]
                # column sums of v over each chunk's fully-masked key
                # blocks, accumulated in PSUM: suf[:, h, c] = sum over
                # kb >= 4c+4 of (v_kb^T @ 1).  Applied (scaled by
                # exp(-10)) as a bias when copying y out of PSUM.  Lives
                # briefly in a pair-pool buffer (read out immediately).
                sufh = {}

                def emit_suffix(b=b, sufh=sufh):
                    suf_ps = pp.tile([65, 2, _NQC - 1], f32, tag="pair",
                                     bufs=2)
                    for h in range(2):
                        for c in range(_NQC - 1):
                            for kb in range(4 * c + 4, _NKB):
                                nc.tensor.matmul(
                                    suf_ps[:, h, c:c + 1],
                                    vaug[:, b * 16 + kb, h, :],
                                    ones_b[:],
                                    start=(kb == 4 * c + 4),
                                    stop=(kb == _NKB - 1),
                                )
                    suf_sb = sp.tile([65, 2, _NQC - 1], f32, tag="sufsb",
                                     bufs=2)
                    nc.scalar.activation(suf_sb[:], suf_ps[:], AF.Copy,
                                         scale=_EXPM)
                    sufh["sb"] = suf_sb

                for c in range(_NQC):
                    cc = slice(c * _TC, (c + 1) * _TC)
                    # both heads run zipped through the pair loop with
                    # separate PSUM accumulators, so every dependency
                    # (exp/affine/copy) hides behind the other head's work
                    yps = [pp.tile([65, _TC], f32, tag="yT", bufs=2,
                                   name=f"yps{h}") for h in range(2)]
                    kbs = list(range(4 * c + 4))

                    def pv_mms(pair, px, c=c, b=b):
                        for h in range(2):
                            for i, kb in enumerate(pair):
                                nc.tensor.matmul(
                                    yps[h][:],
                                    vaug[:, b * 16 + kb, h, :],
                                    px[h][:, i * _TC:(i + 1) * _TC],
                                    start=(kb == 0),
                                    stop=(kb == 4 * c + 3),
                                )

                    # PV matmuls run one pair behind the score matmuls so
                    # the scores->exp->affine->PV latency is hidden behind
                    # the next pair's scores + pumped work
                    prev_pv = None
                    for p0 in range(0, len(kbs), 2):
                        pair = kbs[p0:p0 + 2]
                        pt = [pp.tile([128, 2 * _TC], f32, tag="pair",
                                      bufs=2, name=f"pt{h}")
                              for h in range(2)]
                        px = [sp.tile([128, 2 * _TC], bf16, tag="pexp",
                                      bufs=12, name=f"px{h}")
                              for h in range(2)]
                        offs = [128 * (kb - 4 * c) if kb - 4 * c > 0 else 0
                                for kb in pair]
                        e0 = offs[0]
                        for h in range(2):
                            rows = slice(h * 64, (h + 1) * 64)
                            for i, kb in enumerate(pair):
                                off = offs[i]
                                nc.tensor.matmul(
                                    pt[h][:, i * _TC + off:(i + 1) * _TC],
                                    kT[rows,
                                       b * _T + kb * 128:
                                       b * _T + (kb + 1) * 128],
                                    qT[rows,
                                       b * _T + c * _TC + off:
                                       b * _T + (c + 1) * _TC],
                                    start=True, stop=True,
                                )
                            # exp emitted per head right behind its scores
                            # so the activation engine starts while the
                            # other head's scores still run; the gap in a
                            # j0+j1 band pair is exp'd as garbage (cheaper
                            # than a second exp) and overwritten by the
                            # affine_select fill
                            if e0 >= 256:
                                # j2+j3 band pair: the 384-column gap costs
                                # more than a second instruction
                                nc.scalar.activation(
                                    px[h][:, e0:_TC], pt[h][:, e0:_TC],
                                    AF.Exp, scale=0.125
                                )
                                o1 = _TC + offs[1]
                                nc.scalar.activation(
                                    px[h][:, o1:], pt[h][:, o1:],
                                    AF.Exp, scale=0.125
                                )
                            else:
                                nc.scalar.activation(
                                    px[h][:, e0:], pt[h][:, e0:], AF.Exp,
                                    scale=0.125
                                )
                            for i, kb in enumerate(pair):
                                j = kb - 4 * c
                                if j >= 0:
                                    # causal: keep where qi - ki - 128j
                                    # >= 0, else fill exp(-10)
                                    w = 128 * (j + 1)
                                    nc.gpsimd.affine_select(
                                        out=px[h][:, i * _TC:i * _TC + w],
                                        in_=px[h][:, i * _TC:i * _TC + w],
                                        compare_op=mybir.AluOpType.is_ge,
                                        fill=_EXPM,
                                        base=-128 * j,
                                        pattern=[[1, w]],
                                        channel_multiplier=-1,
                                    )
                        pump()
                        if "sb" not in sufh:
                            emit_suffix()
                        if prev_pv is not None:
                            pv_mms(*prev_pv)
                        prev_pv = (pair, px)
                    pump()
                    pv_mms(*prev_pv)
                    suf_sb = sufh["sb"]
                    for h in range(2):
                        if c < _NQC - 1:
                            nc.scalar.activation(
                                ytmp[h][:, cc], yps[h][:], AF.Identity,
                                bias=suf_sb[:, h, c:c + 1],
                            )
                        else:
                            nc.scalar.copy(ytmp[h][:, cc], yps[h][:])
                    pump()
                    # normalize both heads: reciprocal of the denominator
                    # row, broadcast across the 64 dims on the Pool engine
                    # (partition-stride-0 read), then multiply
                    for h in range(2):
                        rows = slice(h * 64, (h + 1) * 64)
                        zr1 = sp.tile([1, _TC], f32, tag="zr1", bufs=2)
                        nc.vector.reciprocal(zr1[:], ytmp[h][64:65, cc])
                        zb = sp.tile([64, _TC], f32, tag="zrec", bufs=2)
                        nc.gpsimd.partition_broadcast(zb[:], zr1[:])
                        nc.vector.tensor_mul(
                            ycat[rows, cc], ytmp[h][0:64, cc], zb[:]
                        )
                        pump()
                    ostg = sp.tile([128, _KC, _TC], bf16, tag="ostg",
                                   bufs=6)
                    for m in range(_KC):
                        pending.append((b, c, m, ycat, ostg, pump_n[0]))

            drain_pending(len(pending), final=True)

    nc.compile()
    return nc, outT_d.name


def _get_nc():
    if "nc" not in _cache:
        _cache["nc"] = _build()
    return _cache["nc"]


def kernel(**inputs):
    import ml_dtypes

    from concourse.bass_utils import run_bass_kernel_spmd

    bf = ml_dtypes.bfloat16
    x = np.ascontiguousarray(np.asarray(inputs["x"]), dtype=np.float32)
    wq = np.ascontiguousarray(np.asarray(inputs["wq"]), dtype=np.float32)
    wk = np.ascontiguousarray(np.asarray(inputs["wk"]), dtype=np.float32)
    wv = np.ascontiguousarray(np.asarray(inputs["wv"]), dtype=np.float32)
    wo = np.ascontiguousarray(np.asarray(inputs["wo"]), dtype=np.float32)

    xT = np.ascontiguousarray(x.reshape(_BT, _C).T.astype(bf))
    identc = np.zeros((128, 64), dtype=np.float32)
    identc[np.arange(128), np.arange(128) % 64] = 1.0
    onesb = np.ones((128, 1), dtype=bf)

    in_maps = []
    for i in range(_NC):
        r = slice(_LOC * i, _LOC * (i + 1))
        in_maps.append({
            "xT": xT,
            "wqT": np.ascontiguousarray(wq[r].T.astype(bf)),
            "wkT": np.ascontiguousarray(wk[r].T.astype(bf)),
            "wvT": np.ascontiguousarray(wv[r].T.astype(bf)),
            "woT": np.ascontiguousarray(wo[:, r].T),
            "identc": identc,
            "onesb": onesb,
        })

    nc, outname = _get_nc()
    try:
        res = run_bass_kernel_spmd(nc, in_maps, list(range(_NC)), trace=TRACE)
    except ModuleNotFoundError:
        # NTFF profiling hook unavailable in this container
        res = run_bass_kernel_spmd(nc, in_maps, list(range(_NC)), trace=False)

    global LAST_EXEC_NS, LAST_RESULTS
    LAST_EXEC_NS = res.exec_time_ns
    LAST_RESULTS = res

    acc = np.zeros((_C, _BT), dtype=np.float64)
    for i in range(_NC):
        acc += res.results[i][outname].astype(np.float64)
    return np.ascontiguousarray(acc.T).reshape(_B, _T, _C).astype(np.float32)


# revision 103
# speedup vs baseline: 1.0288x; 1.0127x over previous
"""Causal self-attention (B=4, T=2048, C=1024, 16 heads) on 8 TRN2 NeuronCores.

Sharding: tensor-parallel over heads. Each core owns 2 heads (128 of the
1024 q/k/v dims): wq/wk/wv are split by rows (output dim), wo by columns.
Each core computes a full [C, B*T] partial of the output projection; the
host sums the 8 partials (partials are written bf16, summed in f64).

On-core layout is "transposed": activations live as [feature, token] so
every matmul has tokens on the moving free dim. x and the q/k/v weights
are loaded as bf16 (host-side cast), so projections run as bf16 matmuls
(1 cycle/row); q/k are stored bf16, which lets the diagonal-band score
matmuls shrink to their unmasked tail (bf16 has no 256-column floor like
f32r). Attention is computed as s^T = K Q^T with keys on partitions;
softmax max-subtraction is skipped (logits are O(10), exp is safe in
fp32) and the denominator comes from a ones-column appended to V in the
P^T @ V matmul. Causal masking replaces masked probabilities with
exp(-10) (the module masks logits with -10, not -inf); fully-masked key
blocks are skipped and their exp(-10)*colsum(V) contribution enters as a
per-chunk bias. Normalization takes the reciprocal of the denominator
row (DVE), broadcasts it across partitions on the gpsimd engine
(partition_broadcast), and multiplies - no PE broadcast matmul.

Scheduling (tuned against the instruction-level timeline model):
- Score blocks are processed in PAIRS sharing one 2-bank PSUM tile so
  each exp covers ~1024 columns (halves activation instruction count);
  exp'd garbage in band-pair gaps is overwritten by the affine_select
  fill.  The j2+j3 band pair keeps two exps (its gap is wider than the
  instruction overhead).
- Both heads run zipped through the pair loop with separate PSUM
  accumulators; each head's exp is emitted right behind its scores so
  the activation engine starts one head while the other's scores run.
- PV matmuls trail the scores by one pair, hiding the
  scores->exp->affine->PV dependency latency.
- Projections are split into per-chunk units (dma prefetch, q, k, v,
  v-transpose) and drained one at a time into the previous batch's
  attention pair loop ("pump"), as are the deferred output-projection
  matmuls, so the PE never waits long on the exp pipeline.
- Output partials are staged bf16 and written with one batched DMA per
  (batch, chunk); x is loaded with one DMA per [128 x 4 x 512] half; the
  final batch alternates staging copies between DVE and Act and splits
  its DMAs so the tail drain parallelizes.
"""

import os
import sys
from collections import deque

import numpy as np

for _p in ("/opt/trn_rl_repo",):
    if _p not in sys.path and os.path.isdir(_p):
        sys.path.insert(0, _p)

_B, _T, _C = 4, 2048, 1024
_NHEAD, _HD = 16, 64
_NC = 8
_LOC = (_NHEAD // _NC) * _HD  # feature dims per core = 128 (2 heads)
_BT = _B * _T                 # 8192 tokens
_TC = 512                     # token chunk (psum bank / moving-operand width)
_NTC = _BT // _TC             # 16 projection chunks
_KC = _C // 128               # 8 contraction chunks over the embedding
_NQC = _T // _TC              # 4 query chunks per batch
_NKB = _T // 128              # 16 key blocks per batch
_EXPM = float(np.exp(-10.0))  # exp of the mask fill value

TRACE = bool(int(os.environ.get("KERNEL_TRACE", "0")))
LAST_EXEC_NS = None
LAST_RESULTS = None

_cache = {}


def _build():
    import concourse.mybir as mybir
    import concourse.tile as tile
    from concourse import bacc

    f32 = mybir.dt.float32
    f32r = mybir.dt.float32r
    bf16 = mybir.dt.bfloat16
    AF = mybir.ActivationFunctionType

    nc = bacc.Bacc("TRN2", target_bir_lowering=False, debug=False)

    xT_d = nc.dram_tensor("xT", [_C, _BT], bf16, kind="ExternalInput").ap()
    wqT_d = nc.dram_tensor("wqT", [_C, _LOC], bf16, kind="ExternalInput").ap()
    wkT_d = nc.dram_tensor("wkT", [_C, _LOC], bf16, kind="ExternalInput").ap()
    wvT_d = nc.dram_tensor("wvT", [_C, _LOC], bf16, kind="ExternalInput").ap()
    woT_d = nc.dram_tensor("woT", [_LOC, _C], f32r, kind="ExternalInput").ap()
    idc_d = nc.dram_tensor("identc", [128, 64], f32r, kind="ExternalInput").ap()
    oneb_d = nc.dram_tensor("onesb", [128, 1], bf16, kind="ExternalInput").ap()
    outT_d = nc.dram_tensor("outT", [_C, _BT], bf16, kind="ExternalOutput").ap()

    xT_v = xT_d.rearrange("(c p) n -> p c n", p=128)    # [128, 8, 8192]
    wq_v = wqT_d.rearrange("(c p) m -> p c m", p=128)   # [128, 8, 128]
    wk_v = wkT_d.rearrange("(c p) m -> p c m", p=128)
    wv_v = wvT_d.rearrange("(c p) m -> p c m", p=128)
    wo_v = woT_d.rearrange("p (m n) -> p m n", n=128)   # [128, 8, 128]
    outT_v = outT_d.rearrange("(m p) n -> p m n", p=128)  # [128, 8, 8192]

    with tile.TileContext(nc) as tc:
        with (
            tc.tile_pool(name="consts", bufs=1) as cp,
            tc.tile_pool(name="sb", bufs=2) as sp,
            tc.tile_pool(name="ps", bufs=2, space="PSUM") as pp,
        ):
            # startup order: wq + first x chunk first so the projection
            # matmul chain starts as early as possible
            w_sb = {}
            for nm in ("q", "k", "v"):
                w_sb[nm] = cp.tile([128, _KC, 128], bf16, tag=f"w{nm}",
                                   name=f"w{nm}")
            xa0 = sp.tile([128, 4, _TC], bf16, tag="xa", bufs=3)
            xb0 = sp.tile([128, 4, _TC], bf16, tag="xb", bufs=3)

            nc.sync.dma_start(w_sb["q"][:, 0:1, :], wq_v[:, 0:1, :])
            nc.sync.dma_start(xa0[:, 0:1, :], xT_v[:, 0:1, 0:_TC])
            nc.sync.dma_start(w_sb["q"][:, 1:, :], wq_v[:, 1:, :])
            nc.sync.dma_start(xa0[:, 1:, :], xT_v[:, 1:4, 0:_TC])
            nc.sync.dma_start(xb0[:], xT_v[:, 4:8, 0:_TC])
            nc.sync.dma_start(w_sb["k"][:], wk_v[:])
            nc.sync.dma_start(w_sb["v"][:], wv_v[:])
            wo_sb = cp.tile([128, _KC, 128], f32r, tag="wo")
            nc.sync.dma_start(wo_sb[:], wo_v[:])
            ident = cp.tile([128, 64], f32r, tag="ident")
            nc.sync.dma_start(ident[:], idc_d[:])
            ones_b = cp.tile([128, 1], bf16, tag="onesb")
            nc.sync.dma_start(ones_b[:], oneb_d[:])

            qT = cp.tile([128, _BT], bf16, tag="qT")
            kT = cp.tile([128, _BT], bf16, tag="kT")
            # v in [token, dim] layout per 128-token block, per head, with a
            # trailing ones column (row sums -> softmax denominator)
            vaug = cp.tile([128, _BT // 128, 2, 65], bf16, tag="vaug")
            for h in range(2):
                nc.vector.memset(vaug[:, :, h, 64:65], 1.0)

            # ---------------- q/k/v projections ----------------
            # each chunk is split into 4 units (dma prefetch, q, k, v) that
            # are drained one at a time into the previous batch's attention
            # pair loop, keeping the PE fed while exp/affine/copies run
            def _proj_mm(nm, halves, out_cb, t):
                ps = pp.tile([128, _TC], f32, tag="psC", bufs=2, name="ps")
                for c in range(_KC):
                    nc.tensor.matmul(
                        ps[:],
                        w_sb[nm][:, c, :],
                        halves[c // 4][:, c % 4, :],
                        start=(c == 0),
                        stop=(c == _KC - 1),
                    )
                out_cb(ps, t)

            def _q_out(ps, t):
                nc.vector.tensor_copy(qT[:, t * _TC:(t + 1) * _TC], ps[:])

            def _k_out(ps, t):
                nc.vector.tensor_copy(kT[:, t * _TC:(t + 1) * _TC], ps[:])

            def _v_out(ps, t, st):
                vtc = sp.tile([128, _TC], f32r, tag="vtc", bufs=2)
                nc.vector.tensor_copy(vtc[:], ps[:])
                st["vtc"] = vtc

            def _v_tp(t, st):
                vtc = st["vtc"]
                for h in range(2):
                    tp = pp.tile([128, 4, 64], f32, tag="psC", bufs=2)
                    for s4 in range(4):
                        nc.tensor.transpose(
                            tp[:, s4, :].bitcast(f32r),
                            vtc[h * 64:(h + 1) * 64,
                                s4 * 128:(s4 + 1) * 128],
                            ident[h * 64:(h + 1) * 64, :],
                        )
                    nc.vector.tensor_copy(
                        vaug[:, t * 4:t * 4 + 4, h, 0:64], tp[:]
                    )

            def make_proj_units(t):
                tok = slice(t * _TC, (t + 1) * _TC)
                st = {}

                def dma_u(t=t, tok=tok, st=st):
                    if t == 0:
                        st["h"] = (xa0, xb0)
                    else:
                        xa = sp.tile([128, 4, _TC], bf16, tag="xa", bufs=3,
                                     name="xa")
                        xb = sp.tile([128, 4, _TC], bf16, tag="xb", bufs=3,
                                     name="xb")
                        nc.sync.dma_start(xa[:], xT_v[:, 0:4, tok])
                        nc.sync.dma_start(xb[:], xT_v[:, 4:8, tok])
                        st["h"] = (xa, xb)

                return dma_u, [
                    lambda t=t, st=st: _proj_mm("q", st["h"], _q_out, t),
                    lambda t=t, st=st: _proj_mm("k", st["h"], _k_out, t),
                    lambda t=t, st=st: _proj_mm(
                        "v", st["h"], lambda ps, tt: _v_out(ps, tt, st), t),
                    lambda t=t, st=st: _v_tp(t, st),
                ]

            def enqueue_chunks(ts):
                # x DMAs run two chunks ahead of their consumers (the xa/xb
                # rings hold two chunks)
                dmas, rests = [], []
                for t in ts:
                    d, r = make_proj_units(t)
                    dmas.append(d)
                    rests.append(r)
                # the last chunk's v units drain before its q/k: the next
                # batch's suffix matmuls need the whole vaug, while the q/k
                # consumers (its chunk-3 attention) are far away
                order = [dmas[0], dmas[1]]
                for i, r in enumerate(rests):
                    if i >= len(rests) - 2:
                        order.extend(r[2:4] + r[0:2])
                    else:
                        order.extend(r)
                    if i + 2 < len(dmas):
                        order.append(dmas[i + 2])
                proj_q.extend(order)

            # ---------------- attention + output projection ----------------
            # pending: deferred output-projection matmuls, drained into the
            # PE stream between score and PV matmuls of later chunks
            pending = deque()
            proj_q = deque()
            credit = [0.0]
            pump_n = [0]

            def pump():
                pump_n[0] += 1
                drain_pending(2)
                credit[0] += 0.60
                while credit[0] >= 1.0 and proj_q:
                    proj_q.popleft()()
                    credit[0] -= 1.0

            def drain_pending(k, final=False):
                for _ in range(min(k, len(pending))):
                    # let the normalization that feeds a fresh pending age a
                    # couple of pumps before the PE consumes it
                    if not final and pump_n[0] - pending[0][5] < 4:
                        return
                    b_, c_, m_, ycat_, ostg_, _seq = pending.popleft()
                    cc_ = slice(c_ * _TC, (c_ + 1) * _TC)
                    ops = pp.tile([128, _TC], f32, tag="psC", bufs=2)
                    nc.tensor.matmul(
                        ops[:],
                        wo_sb[:, m_, :],
                        ycat_[:, cc_],
                        start=True, stop=True,
                    )
                    # the final batch alternates DVE/Act staging copies and
                    # splits the DMA so the tail drain parallelizes
                    if final:
                        if m_ % 2 == 0:
                            nc.scalar.copy(ostg_[:, m_, :], ops[:])
                        else:
                            nc.vector.tensor_copy(ostg_[:, m_, :], ops[:])
                        half = _KC // 2
                        if m_ == half - 1:
                            nc.sync.dma_start(
                                outT_v[:, 0:half,
                                       b_ * _T + c_ * _TC:
                                       b_ * _T + (c_ + 1) * _TC],
                                ostg_[:, 0:half, :],
                            )
                        elif m_ == _KC - 1:
                            nc.sync.dma_start(
                                outT_v[:, half:,
                                       b_ * _T + c_ * _TC:
                                       b_ * _T + (c_ + 1) * _TC],
                                ostg_[:, half:, :],
                            )
                        continue
                    nc.vector.tensor_copy(ostg_[:, m_, :], ops[:])
                    if m_ == _KC - 1:
                        nc.sync.dma_start(
                            outT_v[:, :, b_ * _T + c_ * _TC:
                                   b_ * _T + (c_ + 1) * _TC],
                            ostg_[:],
                        )

            enqueue_chunks(range(4))

            for b in range(_B):
                # finish this batch's projections (normally already drained
                # into the previous batch's attention), then queue the next
                # batch's projection units for interleaved draining
                while proj_q:
                    proj_q.popleft()()
                if b + 1 < _B:
                    enqueue_chunks(range(4 * (b + 1), 4 * (b + 1) + 4))
                ycat = sp.tile([128, _T], f32r, tag="ycat", bufs=2)
                ytmp = [sp.tile([65, _T], f32r, tag="ytmp", bufs=2,
                                name=f"ytmp{h}")
                        for h in range(2)]
                # column sums of v over each chunk's fully-masked key
                # blocks, accumulated in PSUM: suf[:, h, c] = sum over
                # kb >= 4c+4 of (v_kb^T @ 1).  Applied (scaled by
                # exp(-10)) as a bias when copying y out of PSUM.  Lives
                # briefly in a pair-pool buffer (read out immediately).
                sufh = {}

                def emit_suffix(b=b, sufh=sufh):
                    suf_ps = pp.tile([65, 2, _NQC - 1], f32, tag="pair",
                                     bufs=2)
                    for h in range(2):
                        for c in range(_NQC - 1):
                            for kb in range(4 * c + 4, _NKB):
                                nc.tensor.matmul(
                                    suf_ps[:, h, c:c + 1],
                                    vaug[:, b * 16 + kb, h, :],
                                    ones_b[:],
                                    start=(kb == 4 * c + 4),
                                    stop=(kb == _NKB - 1),
                                )
                    suf_sb = sp.tile([65, 2, _NQC - 1], f32, tag="sufsb",
                                     bufs=2)
                    nc.scalar.activation(suf_sb[:], suf_ps[:], AF.Copy,
                                         scale=_EXPM)
                    sufh["sb"] = suf_sb

                for c in range(_NQC):
                    cc = slice(c * _TC, (c + 1) * _TC)
                    # both heads run zipped through the pair loop with
                    # separate PSUM accumulators, so every dependency
                    # (exp/affine/copy) hides behind the other head's work
                    yps = [pp.tile([65, _TC], f32, tag="yT", bufs=2,
                                   name=f"yps{h}") for h in range(2)]
                    kbs = list(range(4 * c + 4))

                    def pv_mms(pair, px, c=c, b=b):
                        for h in range(2):
                            for i, kb in enumerate(pair):
                                nc.tensor.matmul(
                                    yps[h][:],
                                    vaug[:, b * 16 + kb, h, :],
                                    px[h][:, i * _TC:(i + 1) * _TC],
                                    start=(kb == 0),
                                    stop=(kb == 4 * c + 3),
                                )

                    # PV matmuls run one pair behind the score matmuls so
                    # the scores->exp->affine->PV latency is hidden behind
                    # the next pair's scores + pumped work
                    prev_pv = None
                    for p0 in range(0, len(kbs), 2):
                        pair = kbs[p0:p0 + 2]
                        pt = [pp.tile([128, 2 * _TC], f32, tag="pair",
                                      bufs=2, name=f"pt{h}")
                              for h in range(2)]
                        px = [sp.tile([128, 2 * _TC], bf16, tag="pexp",
                                      bufs=14, name=f"px{h}")
                              for h in range(2)]
                        offs = [128 * (kb - 4 * c) if kb - 4 * c > 0 else 0
                                for kb in pair]
                        e0 = offs[0]
                        for h in range(2):
                            rows = slice(h * 64, (h + 1) * 64)
                            for i, kb in enumerate(pair):
                                off = offs[i]
                                nc.tensor.matmul(
                                    pt[h][:, i * _TC + off:(i + 1) * _TC],
                                    kT[rows,
                                       b * _T + kb * 128:
                                       b * _T + (kb + 1) * 128],
                                    qT[rows,
                                       b * _T + c * _TC + off:
                                       b * _T + (c + 1) * _TC],
                                    start=True, stop=True,
                                )
                            # exp emitted per head right behind its scores
                            # so the activation engine starts while the
                            # other head's scores still run; the gap in a
                            # j0+j1 band pair is exp'd as garbage (cheaper
                            # than a second exp) and overwritten by the
                            # affine_select fill
                            if e0 >= 256:
                                # j2+j3 band pair: the 384-column gap costs
                                # more than a second instruction
                                nc.scalar.activation(
                                    px[h][:, e0:_TC], pt[h][:, e0:_TC],
                                    AF.Exp, scale=0.125
                                )
                                o1 = _TC + offs[1]
                                nc.scalar.activation(
                                    px[h][:, o1:], pt[h][:, o1:],
                                    AF.Exp, scale=0.125
                                )
                            else:
                                nc.scalar.activation(
                                    px[h][:, e0:], pt[h][:, e0:], AF.Exp,
                                    scale=0.125
                                )
                            for i, kb in enumerate(pair):
                                j = kb - 4 * c
                                if j >= 0:
                                    # causal: keep where qi - ki - 128j
                                    # >= 0, else fill exp(-10)
                                    w = 128 * (j + 1)
                                    nc.gpsimd.affine_select(
                                        out=px[h][:, i * _TC:i * _TC + w],
                                        in_=px[h][:, i * _TC:i * _TC + w],
                                        compare_op=mybir.AluOpType.is_ge,
                                        fill=_EXPM,
                                        base=-128 * j,
                                        pattern=[[1, w]],
                                        channel_multiplier=-1,
                                    )
                        pump()
                        if "sb" not in sufh:
                            emit_suffix()
                        if prev_pv is not None:
                            pv_mms(*prev_pv)
                        prev_pv = (pair, px)
                    pump()
                    pv_mms(*prev_pv)
                    suf_sb = sufh["sb"]
                    for h in range(2):
                        if c < _NQC - 1:
                            nc.scalar.activation(
                                ytmp[h][:, cc], yps[h][:], AF.Identity,
                                bias=suf_sb[:, h, c:c + 1],
                            )
                        else:
                            nc.scalar.copy(ytmp[h][:, cc], yps[h][:])
                    pump()
                    # normalize both heads: reciprocal of the denominator
                    # row, broadcast across the 64 dims on the Pool engine
                    # (partition-stride-0 read), then multiply
                    for h in range(2):
                        rows = slice(h * 64, (h + 1) * 64)
                        zr1 = sp.tile([1, _TC], f32, tag="zr1", bufs=2)
                        nc.vector.reciprocal(zr1[:], ytmp[h][64:65, cc])
                        zb = sp.tile([64, _TC], f32, tag="zrec", bufs=2)
                        nc.gpsimd.partition_broadcast(zb[:], zr1[:])
                        nc.vector.tensor_mul(
                            ycat[rows, cc], ytmp[h][0:64, cc], zb[:]
                        )
                        pump()
                    ostg = sp.tile([128, _KC, _TC], bf16, tag="ostg",
                                   bufs=6)
                    for m in range(_KC):
                        pending.append((b, c, m, ycat, ostg, pump_n[0]))

            drain_pending(len(pending), final=True)

    nc.compile()
    return nc, outT_d.name


def _get_nc():
    if "nc" not in _cache:
        _cache["nc"] = _build()
    return _cache["nc"]


def kernel(**inputs):
    import ml_dtypes

    from concourse.bass_utils import run_bass_kernel_spmd

    bf = ml_dtypes.bfloat16
    x = np.ascontiguousarray(np.asarray(inputs["x"]), dtype=np.float32)
    wq = np.ascontiguousarray(np.asarray(inputs["wq"]), dtype=np.float32)
    wk = np.ascontiguousarray(np.asarray(inputs["wk"]), dtype=np.float32)
    wv = np.ascontiguousarray(np.asarray(inputs["wv"]), dtype=np.float32)
    wo = np.ascontiguousarray(np.asarray(inputs["wo"]), dtype=np.float32)

    xT = np.ascontiguousarray(x.reshape(_BT, _C).T.astype(bf))
    identc = np.zeros((128, 64), dtype=np.float32)
    identc[np.arange(128), np.arange(128) % 64] = 1.0
    onesb = np.ones((128, 1), dtype=bf)

    in_maps = []
    for i in range(_NC):
        r = slice(_LOC * i, _LOC * (i + 1))
        in_maps.append({
            "xT": xT,
            "wqT": np.ascontiguousarray(wq[r].T.astype(bf)),
            "wkT": np.ascontiguousarray(wk[r].T.astype(bf)),
            "wvT": np.ascontiguousarray(wv[r].T.astype(bf)),
            "woT": np.ascontiguousarray(wo[:, r].T),
            "identc": identc,
            "onesb": onesb,
        })

    nc, outname = _get_nc()
    try:
        res = run_bass_kernel_spmd(nc, in_maps, list(range(_NC)), trace=TRACE)
    except ModuleNotFoundError:
        # NTFF profiling hook unavailable in this container
        res = run_bass_kernel_spmd(nc, in_maps, list(range(_NC)), trace=False)

    global LAST_EXEC_NS, LAST_RESULTS
    LAST_EXEC_NS = res.exec_time_ns
    LAST_RESULTS = res

    acc = np.zeros((_C, _BT), dtype=np.float64)
    for i in range(_NC):
        acc += res.results[i][outname].astype(np.float64)
    return np.ascontiguousarray(acc.T).reshape(_B, _T, _C).astype(np.float32)
